# revision 12
# baseline (speedup 1.0000x reference)
"""Causal self-attention kernel for 8 Trainium2 NeuronCores.

Sharding (Megatron-style): core c handles batch b=c//2 and head-group
g=c%2 (8 of 16 heads). qkv projection is column-parallel, out
projection row-parallel; the host sums the two partial y outputs per
batch and adds b_out.

On-device math (per core), all matmuls in float32r (full PE rate):
  xT = x^T                          (PE transpose blocks)
  Q^T = Wq^T xT, K^T = Wk^T xT      (feature-major, [512, 2048])
  V   = x Wv                        (token-major, + ones column per head)
  S^T = K Q^T (per head, row-packed pairs on the PE array)
  P^T = exp(scale*S^T + causal mask)   (no max-subtraction; scores are O(8))
  O^T_unnorm = V_aug^T P^T  (65th row accumulates the softmax denominator Z)
  O^T = O^T_unnorm * broadcast(1/Z)
  y_partial = O^T^T W_out_slice     (token-major)

Attention runs two independent (head-pair, q-chunk) chains interleaved
so ScalarE (exp) and the PE pipeline each other without ping-pong.
"""

import sys

sys.path.insert(0, "/opt/trn_rl_repo")

import numpy as np

from concourse import bacc
import concourse.mybir as mybir
import concourse.tile as tile
from concourse.bass_utils import run_bass_kernel_spmd

F32 = mybir.dt.float32
F32R = mybir.dt.float32r
AF = mybir.ActivationFunctionType

B, T, C = 4, 2048, 1024
NH, HD = 16, 64
HPC = 8          # heads per core
GF = HPC * HD    # 512: per-core q/k/v feature width
NCORES = 8
SCALE = 1.0 / np.sqrt(HD)

PT128 = T // 128   # 16 token tiles
NQC = T // 512     # 4 q-chunks
KT_C = C // 128    # 8 contraction tiles for x projections
NMT = GF // 128    # 4 feature tiles (2 heads each)
VW = HPC * (HD + 1)  # 520: V_aug width

_PROGRAM = None


def _build_body(nc, tc, xb_d, wq_d, wk_d, wv_d, wo_d, bq_d, bk_d, bva_d,
                mk_d, id_d, yp_d, ko_d, vo_d):
    from contextlib import ExitStack
    with ExitStack() as ctx:
        cst = ctx.enter_context(tc.tile_pool(name="cst", bufs=1))
        qtkt = ctx.enter_context(tc.tile_pool(name="qtkt", bufs=1))
        vap = ctx.enter_context(tc.tile_pool(name="vaug", bufs=1))

        # ---- constants ----
        ident = cst.tile([128, 128], F32, tag="ident")
        nc.sync.dma_start(ident[:], id_d[:])
        identr = cst.tile([128, 128], F32R, tag="identr")
        nc.vector.tensor_copy(identr[:], ident[:])
        mkt = cst.tile([128, 128], F32, tag="mkt")
        nc.sync.dma_start(mkt[:], mk_d[:])
        bq_t, bk_t = [], []
        for m in range(NMT):
            t1 = cst.tile([128, 1], F32, tag=f"bq{m}", name=f"bq{m}")
            nc.sync.dma_start(t1[:], bq_d[m * 128:(m + 1) * 128, :])
            bq_t.append(t1)
            t2 = cst.tile([128, 1], F32, tag=f"bk{m}", name=f"bk{m}")
            nc.sync.dma_start(t2[:], bk_d[m * 128:(m + 1) * 128, :])
            bk_t.append(t2)
        bva_row = cst.tile([1, VW], F32, tag="bvarow")
        nc.sync.dma_start(bva_row[:], bva_d[:])
        bvb = cst.tile([128, VW], F32, tag="bvb")
        nc.gpsimd.partition_broadcast(bvb[:], bva_row[:])

        qt_t = [qtkt.tile([128, T], F32R, tag=f"qt{m}", name=f"qt{m}")
                for m in range(NMT)]
        kt_t = [qtkt.tile([128, T], F32R, tag=f"kt{m}", name=f"kt{m}")
                for m in range(NMT)]
        va_t = [vap.tile([128, VW], F32R, tag=f"va{i}", name=f"va{i}")
                for i in range(PT128)]

        # ================= Phase 1: projections (two token halves) ========
        for half in range(2):
            t0 = half * (T // 2)
            ntt = T // 2 // 128
            with tc.tile_pool(name=f"p1s{half}", bufs=3) as stg, \
                 tc.tile_pool(name=f"p1x{half}", bufs=1) as xtp, \
                 tc.tile_pool(name=f"p1w{half}", bufs=1) as wp, \
                 tc.tile_pool(name=f"p1pa{half}", bufs=2,
                              space="PSUM") as psa, \
                 tc.tile_pool(name=f"p1pb{half}", bufs=2,
                              space="PSUM") as psb:
                # prefetch Wq for this half (cast on Pool, during x DMA)
                wq_t = []
                for kt in range(KT_C):
                    ws = stg.tile([128, GF], F32, tag="ws",
                                  name=f"wspre{half}_{kt}")
                    nc.sync.dma_start(ws[:], wq_d[kt * 128:(kt + 1) * 128, :])
                    wr = wp.tile([128, GF], F32R, tag=f"w{kt}",
                                 name=f"wpre{half}_{kt}")
                    nc.gpsimd.tensor_copy(wr[:], ws[:])
                    wq_t.append(wr)

                # ---- x^T for this half ----
                xt_t = [xtp.tile([128, T // 2], F32R, tag=f"xt{fb}",
                                 name=f"xt{half}_{fb}")
                        for fb in range(KT_C)]
                for tt in range(ntt):
                    xs = stg.tile([128, C], F32, tag="xs",
                                  name=f"xs{half}_{tt}")
                    nc.scalar.dma_start(
                        xs[:], xb_d[t0 + tt * 128:t0 + (tt + 1) * 128, :])
                    for fb in range(KT_C):
                        tp = psa.tile([128, 512], F32, tag="tp",
                                      name=f"tp{half}_{tt}_{fb}")
                        nc.tensor.transpose(
                            tp[:, 0:128], xs[:, fb * 128:(fb + 1) * 128],
                            ident[:])
                        nc.vector.tensor_copy(
                            xt_t[fb][:, tt * 128:(tt + 1) * 128],
                            tp[:, 0:128])

                # ---- Q^T and K^T (feature-major) ----
                for which, w_d, bias_t, dst in (
                        ("q", wq_d, bq_t, qt_t), ("k", wk_d, bk_t, kt_t)):
                    if which == "q":
                        w_t = wq_t
                    else:
                        w_t = []
                        for kt in range(KT_C):
                            ws = stg.tile([128, GF], F32, tag="ws",
                                          name=f"ws{half}k_{kt}")
                            nc.sync.dma_start(
                                ws[:], w_d[kt * 128:(kt + 1) * 128, :])
                            wr = wp.tile([128, GF], F32R, tag=f"w{kt}",
                                         name=f"w{half}k_{kt}")
                            nc.gpsimd.tensor_copy(wr[:], ws[:])
                            w_t.append(wr)
                    for m in range(NMT):
                        for n in range(T // 2 // 512):
                            ps = psb.tile([128, 512], F32, tag="pj",
                                          name=f"pj{half}{which}_{m}_{n}")
                            for kt in range(KT_C):
                                nc.tensor.matmul(
                                    ps[:],
                                    w_t[kt][:, m * 128:(m + 1) * 128],
                                    xt_t[kt][:, n * 512:(n + 1) * 512],
                                    start=(kt == 0), stop=(kt == KT_C - 1))
                            nc.vector.tensor_scalar_add(
                                dst[m][:, t0 + n * 512:t0 + (n + 1) * 512],
                                ps[:], bias_t[m][:])

                # ---- V (token-major) + V_aug + v output ----
                w_t = []
                for kt in range(KT_C):
                    ws = stg.tile([128, GF], F32, tag="ws",
                                  name=f"ws{half}v_{kt}")
                    nc.sync.dma_start(ws[:], wv_d[kt * 128:(kt + 1) * 128, :])
                    wr = wp.tile([128, GF], F32R, tag=f"w{kt}",
                                 name=f"w{half}v_{kt}")
                    nc.gpsimd.tensor_copy(wr[:], ws[:])
                    w_t.append(wr)
                for tt in range(ntt):
                    gt = half * ntt + tt
                    ps = psb.tile([128, GF], F32, tag="pj",
                                  name=f"pjv{half}_{tt}")
                    for kt in range(KT_C):
                        nc.tensor.matmul(
                            ps[:], xt_t[kt][:, tt * 128:(tt + 1) * 128],
                            w_t[kt][:], start=(kt == 0),
                            stop=(kt == KT_C - 1))
                    vs = stg.tile([128, GF], F32, tag="vs",
                                  name=f"vs{half}_{tt}")
                    nc.scalar.copy(vs[:], ps[:])
                    nc.scalar.dma_start(
                        vo_d[t0 + tt * 128:t0 + (tt + 1) * 128, :], vs[:])
                    for h in range(HPC):
                        nc.vector.tensor_add(
                            va_t[gt][:, h * 65:h * 65 + 64],
                            ps[:, h * 64:(h + 1) * 64],
                            bvb[:, h * 65:h * 65 + 64])
                    nc.gpsimd.tensor_copy(
                        va_t[gt][:].rearrange(
                            "p (h c) -> p h c", c=65)[:, :, 64:65],
                        bvb[:].rearrange(
                            "p (h c) -> p h c", c=65)[:, :, 64:65])

                # ---- k output: transpose K^T back to token-major ----
                for tt in range(ntt):
                    ks = stg.tile([128, GF], F32, tag="ks",
                                  name=f"ks{half}_{tt}")
                    for m in range(NMT):
                        tp = psa.tile([128, 512], F32, tag="tp",
                                      name=f"ktp{half}_{m}_{tt}")
                        nc.tensor.transpose(
                            tp[:, 0:128].bitcast(F32R),
                            kt_t[m][:, t0 + tt * 128:t0 + (tt + 1) * 128],
                            identr[:])
                        nc.vector.tensor_copy(
                            ks[:, m * 128:(m + 1) * 128],
                            tp[:, 0:128].bitcast(F32R))
                    nc.scalar.dma_start(
                        ko_d[t0 + tt * 128:t0 + (tt + 1) * 128, :], ks[:])

        # ---- prefetch + cast W_out early (Pool/DMA idle in P2) ----
        wp3 = ctx.enter_context(tc.tile_pool(name="p3w", bufs=1))
        wo_t = []
        with tc.tile_pool(name="wostg", bufs=2) as wostg:
            for pt_i in range(NMT):
                ws = wostg.tile([128, C], F32, tag="ws3", name=f"wo_s{pt_i}")
                nc.sync.dma_start(ws[:], wo_d[pt_i * 128:(pt_i + 1) * 128, :])
                wr = wp3.tile([128, C], F32R, tag=f"wo{pt_i}",
                              name=f"wo_r{pt_i}")
                nc.gpsimd.tensor_copy(wr[:], ws[:])
                wo_t.append(wr)

        # ================= Phase 2: attention =============================
        otp = ctx.enter_context(tc.tile_pool(name="ot", bufs=1))
        ot_t = [otp.tile([128, T], F32R, tag=f"ot{m}", name=f"ot{m}")
                for m in range(NMT)]
        with tc.tile_pool(name="p2pt", bufs=2) as ptp, \
             tc.tile_pool(name="p2n", bufs=1) as nrm, \
             tc.tile_pool(name="p2y", bufs=2) as stgy, \
             tc.tile_pool(name="p2s", bufs=1, space="PSUM") as sbp, \
             tc.tile_pool(name="p2o", bufs=1, space="PSUM") as obp:
            for j in range(NQC):
                nkt = 4 * (j + 1)
                for chains in ((0, 1), (2, 3)):
                    o_ps = {}
                    for c, pr in enumerate(chains):
                        for h in range(2):
                            o_ps[(pr, h)] = obp.tile(
                                [65, 512], F32, tag=f"o{c}_{h}",
                                name=f"o{pr}_{j}_{h}")

                    def emit_pv(pr, pt, kt):
                        for h in range(2):
                            nc.tensor.matmul(
                                o_ps[(pr, h)][:],
                                va_t[kt][:, (2 * pr + h) * 65:
                                         (2 * pr + h) * 65 + 65],
                                pt[:, h * 512:(h + 1) * 512],
                                start=(kt == 0), stop=(kt == nkt - 1))

                    pend = {pr: None for pr in chains}
                    for kt in range(nkt):
                        band = kt - 4 * j >= 0
                        for c, pr in enumerate(chains):
                            sbig = sbp.tile([128, 1024], F32, tag=f"s{c}",
                                            name=f"s{pr}_{j}_{kt}")
                            for h in range(2):
                                pb = 64 * h
                                nc.tensor.matmul(
                                    sbig[:, h * 512:(h + 1) * 512],
                                    kt_t[pr][pb:pb + 64,
                                             kt * 128:(kt + 1) * 128],
                                    qt_t[pr][pb:pb + 64,
                                             j * 512:(j + 1) * 512],
                                    start=True, stop=True,
                                    tile_position=(pb, 0))
                            if band:
                                o = 128 * (kt - 4 * j)
                                for h in range(2):
                                    nc.vector.tensor_add(
                                        sbig[:, h * 512 + o:h * 512 + o + 128],
                                        sbig[:, h * 512 + o:h * 512 + o + 128],
                                        mkt[:])
                            pt = ptp.tile([128, 1024], F32R, tag=f"pt{c}",
                                          name=f"pt{pr}_{j}_{kt}")
                            nc.scalar.activation(pt[:], sbig[:],
                                                 AF.Exp, scale=SCALE)
                            if band and kt - 4 * j > 0:
                                o = 128 * (kt - 4 * j)
                                for h in range(2):
                                    nc.vector.memset(
                                        pt[:, h * 512:h * 512 + o].bitcast(F32),
                                        0.0)
                            if pend[pr] is not None:
                                emit_pv(pr, *pend[pr])
                            pend[pr] = (pt, kt)
                    for pr in chains:
                        emit_pv(pr, *pend[pr])
                    # normalize rows by Z (row 64) and store to OT
                    for c, pr in enumerate(chains):
                        for h in range(2):
                            rr = nrm.tile([1, 512], F32, tag=f"rr{c}{h}",
                                          name=f"rr{pr}_{j}_{h}")
                            nc.vector.reciprocal(rr[:], o_ps[(pr, h)][64:65, :])
                            bc = nrm.tile([64, 512], F32, tag=f"bc{c}{h}",
                                          name=f"bc{pr}_{j}_{h}")
                            nc.gpsimd.partition_broadcast(bc[:], rr[:])
                            nc.vector.tensor_mul(
                                ot_t[pr][h * 64:(h + 1) * 64,
                                         j * 512:(j + 1) * 512],
                                o_ps[(pr, h)][0:64, :], bc[:])
                # out-projection for this q-chunk (reuses O psum banks)
                for ti, tt in enumerate(range(4 * j, 4 * j + 4)):
                    ys = stgy.tile([128, C], F32, tag="ys", name=f"ys{tt}")
                    for ncol in range(2):
                        oc, oh = divmod((ti * 2 + ncol) % 4, 2)
                        ps = obp.tile([128, 512], F32, tag=f"o{oc}_{oh}",
                                      name=f"py{tt}_{ncol}")
                        for pt_i in range(NMT):
                            nc.tensor.matmul(
                                ps[:], ot_t[pt_i][:, tt * 128:(tt + 1) * 128],
                                wo_t[pt_i][:, ncol * 512:(ncol + 1) * 512],
                                start=(pt_i == 0), stop=(pt_i == NMT - 1))
                        nc.scalar.copy(ys[:, ncol * 512:(ncol + 1) * 512],
                                       ps[:])
                    nc.sync.dma_start(yp_d[tt * 128:(tt + 1) * 128, :], ys[:])





def _build_program():
    nc = bacc.Bacc()
    xb_d = nc.dram_tensor("xb", [T, C], F32, kind="ExternalInput")
    wq_d = nc.dram_tensor("wq", [C, GF], F32, kind="ExternalInput")
    wk_d = nc.dram_tensor("wk", [C, GF], F32, kind="ExternalInput")
    wv_d = nc.dram_tensor("wv", [C, GF], F32, kind="ExternalInput")
    wo_d = nc.dram_tensor("wo", [GF, C], F32, kind="ExternalInput")
    bq_d = nc.dram_tensor("bq", [GF, 1], F32, kind="ExternalInput")
    bk_d = nc.dram_tensor("bk", [GF, 1], F32, kind="ExternalInput")
    bva_d = nc.dram_tensor("bva", [1, VW], F32, kind="ExternalInput")
    mk_d = nc.dram_tensor("mk", [128, 128], F32, kind="ExternalInput")
    id_d = nc.dram_tensor("ident", [128, 128], F32, kind="ExternalInput")
    yp_d = nc.dram_tensor("yp", [T, C], F32, kind="ExternalOutput")
    ko_d = nc.dram_tensor("ko", [T, GF], F32, kind="ExternalOutput")
    vo_d = nc.dram_tensor("vo", [T, GF], F32, kind="ExternalOutput")

    with tile.TileContext(nc) as tc:
        _build_body(nc, tc, xb_d, wq_d, wk_d, wv_d, wo_d, bq_d, bk_d, bva_d,
                    mk_d, id_d, yp_d, ko_d, vo_d)
    nc.finalize()
    return nc


def _get_program():
    global _PROGRAM
    if _PROGRAM is None:
        _PROGRAM = _build_program()
    return _PROGRAM


def _make_in_maps(x, W_qkv, b_qkv, W_out, b_out):
    kk = np.arange(128)[:, None]
    qq = np.arange(128)[None, :]
    mk = np.where(kk > qq, np.float32(-1e30), np.float32(0.0))
    ident = np.eye(128, dtype=np.float32)

    in_maps = []
    for c in range(NCORES):
        b, g = divmod(c, 2)
        cs = slice(g * GF, (g + 1) * GF)
        bva = np.zeros((1, VW), dtype=np.float32)
        bv = b_qkv[2 * C + g * GF:2 * C + (g + 1) * GF]
        for h in range(HPC):
            bva[0, h * 65:h * 65 + 64] = bv[h * 64:(h + 1) * 64]
            bva[0, h * 65 + 64] = 1.0
        in_maps.append({
            "xb": np.ascontiguousarray(x[b]),
            "wq": np.ascontiguousarray(W_qkv[:, cs]),
            "wk": np.ascontiguousarray(W_qkv[:, C:][:, cs]),
            "wv": np.ascontiguousarray(W_qkv[:, 2 * C:][:, cs]),
            "wo": np.ascontiguousarray(W_out[cs, :]),
            "bq": np.ascontiguousarray(b_qkv[cs].reshape(GF, 1)),
            "bk": np.ascontiguousarray(b_qkv[C:][cs].reshape(GF, 1)),
            "bva": bva,
            "mk": mk,
            "ident": ident,
        })
    return in_maps


def run(x, W_qkv, b_qkv, W_out, b_out, trace=False):
    x = np.asarray(x, dtype=np.float32)
    W_qkv = np.asarray(W_qkv, dtype=np.float32)
    b_qkv = np.asarray(b_qkv, dtype=np.float32)
    W_out = np.asarray(W_out, dtype=np.float32)
    b_out = np.asarray(b_out, dtype=np.float32)

    nc = _get_program()
    in_maps = _make_in_maps(x, W_qkv, b_qkv, W_out, b_out)
    res = run_bass_kernel_spmd(nc, in_maps, core_ids=list(range(NCORES)),
                               trace=trace)

    y = np.empty((B, T, C), dtype=np.float32)
    k = np.empty((B, NH, T, HD), dtype=np.float32)
    v = np.empty((B, NH, T, HD), dtype=np.float32)
    for b in range(B):
        r0, r1 = res.results[2 * b], res.results[2 * b + 1]
        y[b] = r0["yp"] + r1["yp"] + b_out[None, :]
        for g, r in ((0, r0), (1, r1)):
            k[b, g * HPC:(g + 1) * HPC] = np.moveaxis(
                r["ko"].reshape(T, HPC, HD), 1, 0)
            v[b, g * HPC:(g + 1) * HPC] = np.moveaxis(
                r["vo"].reshape(T, HPC, HD), 1, 0)
    return (y, k, v), res


def kernel(x, W_qkv, b_qkv, W_out, b_out):
    (y, k, v), _ = run(x, W_qkv, b_qkv, W_out, b_out, trace=False)
    return (y, k, v)


# revision 15
# speedup vs baseline: 1.0573x; 1.0573x over previous
"""Causal self-attention kernel for 8 Trainium2 NeuronCores.

Sharding (Megatron-style): core c handles batch b=c//2 and head-group
g=c%2 (8 of 16 heads). qkv projection is column-parallel, out
projection row-parallel; the host sums the two partial y outputs per
batch and adds b_out.

On-device math (per core), all matmuls in float32r (full PE rate):
  xT = x^T                          (PE transpose blocks)
  Q^T = Wq^T xT, K^T = Wk^T xT      (feature-major, [512, 2048])
  V   = x Wv                        (token-major, + ones column per head)
  S^T = K Q^T (per head, row-packed pairs on the PE array)
  P^T = exp(scale*S^T + causal mask)   (no max-subtraction; scores are O(8))
  O^T_unnorm = V_aug^T P^T  (65th row accumulates the softmax denominator Z)
  O^T = O^T_unnorm * broadcast(1/Z)
  y_partial = O^T^T W_out_slice     (token-major)

Attention runs two independent (head-pair, q-chunk) chains interleaved
so ScalarE (exp) and the PE pipeline each other without ping-pong.
"""

import sys

sys.path.insert(0, "/opt/trn_rl_repo")

import numpy as np

from concourse import bacc
import concourse.mybir as mybir
import concourse.tile as tile
from concourse.bass_utils import run_bass_kernel_spmd

F32 = mybir.dt.float32
F32R = mybir.dt.float32r
AF = mybir.ActivationFunctionType

B, T, C = 4, 2048, 1024
NH, HD = 16, 64
HPC = 8          # heads per core
GF = HPC * HD    # 512: per-core q/k/v feature width
NCORES = 8
SCALE = 1.0 / np.sqrt(HD)

PT128 = T // 128   # 16 token tiles
NQC = T // 512     # 4 q-chunks
KT_C = C // 128    # 8 contraction tiles for x projections
NMT = GF // 128    # 4 feature tiles (2 heads each)
VW = HPC * (HD + 1)  # 520: V_aug width

_PROGRAM = None


def _build_body(nc, tc, xb_d, wq_d, wk_d, wv_d, wo_d, bq_d, bk_d, bva_d,
                mk_d, id_d, yp_d, ko_d, vo_d):
    from contextlib import ExitStack
    with ExitStack() as ctx:
        cst = ctx.enter_context(tc.tile_pool(name="cst", bufs=1))
        qtkt = ctx.enter_context(tc.tile_pool(name="qtkt", bufs=1))
        vap = ctx.enter_context(tc.tile_pool(name="vaug", bufs=1))

        # ---- constants ----
        ident = cst.tile([128, 128], F32, tag="ident")
        nc.sync.dma_start(ident[:], id_d[:])
        identr = cst.tile([128, 128], F32R, tag="identr")
        nc.vector.tensor_copy(identr[:], ident[:])
        mkt = cst.tile([128, 128], F32, tag="mkt")
        nc.sync.dma_start(mkt[:], mk_d[:])
        bq_t, bk_t = [], []
        for m in range(NMT):
            t1 = cst.tile([128, 1], F32, tag=f"bq{m}", name=f"bq{m}")
            nc.sync.dma_start(t1[:], bq_d[m * 128:(m + 1) * 128, :])
            bq_t.append(t1)
            t2 = cst.tile([128, 1], F32, tag=f"bk{m}", name=f"bk{m}")
            nc.sync.dma_start(t2[:], bk_d[m * 128:(m + 1) * 128, :])
            bk_t.append(t2)
        bva_row = cst.tile([1, VW], F32, tag="bvarow")
        nc.sync.dma_start(bva_row[:], bva_d[:])
        bvb = cst.tile([128, VW], F32, tag="bvb")
        nc.gpsimd.partition_broadcast(bvb[:], bva_row[:])

        qt_t = [qtkt.tile([128, T], F32R, tag=f"qt{m}", name=f"qt{m}")
                for m in range(NMT)]
        kt_t = [qtkt.tile([128, T], F32R, tag=f"kt{m}", name=f"kt{m}")
                for m in range(NMT)]
        va_t = [vap.tile([128, VW], F32R, tag=f"va{i}", name=f"va{i}")
                for i in range(PT128)]

        # ================= Phase 1: projections (two token halves) ========
        for half in range(2):
            t0 = half * (T // 2)
            ntt = T // 2 // 128
            with tc.tile_pool(name=f"p1s{half}", bufs=4) as stg, \
                 tc.tile_pool(name=f"p1x{half}", bufs=1) as xtp, \
                 tc.tile_pool(name=f"p1w{half}", bufs=1) as wp, \
                 tc.tile_pool(name=f"p1pa{half}", bufs=2,
                              space="PSUM") as psa, \
                 tc.tile_pool(name=f"p1pb{half}", bufs=2,
                              space="PSUM") as psb:
                # prefetch Wq for this half (cast on Pool, during x DMA)
                wq_t = []
                for kt in range(KT_C):
                    ws = stg.tile([128, GF], F32, tag="ws",
                                  name=f"wspre{half}_{kt}")
                    nc.sync.dma_start(ws[:], wq_d[kt * 128:(kt + 1) * 128, :])
                    wr = wp.tile([128, GF], F32R, tag=f"w{kt}",
                                 name=f"wpre{half}_{kt}")
                    nc.gpsimd.tensor_copy(wr[:], ws[:])
                    wq_t.append(wr)

                # ---- x^T for this half ----
                xt_t = [xtp.tile([128, T // 2], F32R, tag=f"xt{fb}",
                                 name=f"xt{half}_{fb}")
                        for fb in range(KT_C)]
                for tt in range(ntt):
                    xs = stg.tile([128, C], F32, tag="xs",
                                  name=f"xs{half}_{tt}")
                    nc.scalar.dma_start(
                        xs[:], xb_d[t0 + tt * 128:t0 + (tt + 1) * 128, :])
                    for fb in range(KT_C):
                        tp = psa.tile([128, 512], F32, tag="tp",
                                      name=f"tp{half}_{tt}_{fb}")
                        nc.tensor.transpose(
                            tp[:, 0:128], xs[:, fb * 128:(fb + 1) * 128],
                            ident[:])
                        nc.vector.tensor_copy(
                            xt_t[fb][:, tt * 128:(tt + 1) * 128],
                            tp[:, 0:128])

                # ---- Q^T and K^T (feature-major) ----
                for which, w_d, bias_t, dst in (
                        ("q", wq_d, bq_t, qt_t), ("k", wk_d, bk_t, kt_t)):
                    if which == "q":
                        w_t = wq_t
                    else:
                        w_t = []
                        for kt in range(KT_C):
                            ws = stg.tile([128, GF], F32, tag="ws",
                                          name=f"ws{half}k_{kt}")
                            nc.sync.dma_start(
                                ws[:], w_d[kt * 128:(kt + 1) * 128, :])
                            wr = wp.tile([128, GF], F32R, tag=f"w{kt}",
                                         name=f"w{half}k_{kt}")
                            nc.gpsimd.tensor_copy(wr[:], ws[:])
                            w_t.append(wr)
                    for m in range(NMT):
                        for n in range(T // 2 // 512):
                            ps = psb.tile([128, 512], F32, tag="pj",
                                          name=f"pj{half}{which}_{m}_{n}")
                            for kt in range(KT_C):
                                nc.tensor.matmul(
                                    ps[:],
                                    w_t[kt][:, m * 128:(m + 1) * 128],
                                    xt_t[kt][:, n * 512:(n + 1) * 512],
                                    start=(kt == 0), stop=(kt == KT_C - 1))
                            nc.vector.tensor_scalar_add(
                                dst[m][:, t0 + n * 512:t0 + (n + 1) * 512],
                                ps[:], bias_t[m][:])

                # ---- V (token-major) + V_aug + v output ----
                w_t = []
                for kt in range(KT_C):
                    ws = stg.tile([128, GF], F32, tag="ws",
                                  name=f"ws{half}v_{kt}")
                    nc.sync.dma_start(ws[:], wv_d[kt * 128:(kt + 1) * 128, :])
                    wr = wp.tile([128, GF], F32R, tag=f"w{kt}",
                                 name=f"w{half}v_{kt}")
                    nc.gpsimd.tensor_copy(wr[:], ws[:])
                    w_t.append(wr)
                for tt in range(ntt):
                    gt = half * ntt + tt
                    ps = psb.tile([128, GF], F32, tag="pj",
                                  name=f"pjv{half}_{tt}")
                    for kt in range(KT_C):
                        nc.tensor.matmul(
                            ps[:], xt_t[kt][:, tt * 128:(tt + 1) * 128],
                            w_t[kt][:], start=(kt == 0),
                            stop=(kt == KT_C - 1))
                    vs = stg.tile([128, GF], F32, tag="vs",
                                  name=f"vs{half}_{tt}")
                    nc.scalar.copy(vs[:], ps[:])
                    nc.scalar.dma_start(
                        vo_d[t0 + tt * 128:t0 + (tt + 1) * 128, :], vs[:])
                    for h in range(HPC):
                        nc.vector.tensor_add(
                            va_t[gt][:, h * 65:h * 65 + 64],
                            ps[:, h * 64:(h + 1) * 64],
                            bvb[:, h * 65:h * 65 + 64])
                    nc.gpsimd.tensor_copy(
                        va_t[gt][:].rearrange(
                            "p (h c) -> p h c", c=65)[:, :, 64:65],
                        bvb[:].rearrange(
                            "p (h c) -> p h c", c=65)[:, :, 64:65])

                # ---- k output: transpose K^T back to token-major ----
                for tt in range(ntt):
                    ks = stg.tile([128, GF], F32, tag="ks",
                                  name=f"ks{half}_{tt}")
                    for m in range(NMT):
                        tp = psa.tile([128, 512], F32, tag="tp",
                                      name=f"ktp{half}_{m}_{tt}")
                        nc.tensor.transpose(
                            tp[:, 0:128].bitcast(F32R),
                            kt_t[m][:, t0 + tt * 128:t0 + (tt + 1) * 128],
                            identr[:])
                        nc.vector.tensor_copy(
                            ks[:, m * 128:(m + 1) * 128],
                            tp[:, 0:128].bitcast(F32R))
                    nc.scalar.dma_start(
                        ko_d[t0 + tt * 128:t0 + (tt + 1) * 128, :], ks[:])


        # ---- prefetch + cast W_out early (Pool/DMA idle in P2) ----
        wp3 = ctx.enter_context(tc.tile_pool(name="p3w", bufs=1))
        wo_t = []
        with tc.tile_pool(name="wostg", bufs=2) as wostg:
            for pt_i in range(NMT):
                ws = wostg.tile([128, C], F32, tag="ws3", name=f"wo_s{pt_i}")
                nc.sync.dma_start(ws[:], wo_d[pt_i * 128:(pt_i + 1) * 128, :])
                wr = wp3.tile([128, C], F32R, tag=f"wo{pt_i}",
                              name=f"wo_r{pt_i}")
                nc.gpsimd.tensor_copy(wr[:], ws[:])
                wo_t.append(wr)

        # ================= Phase 2: attention =============================
        otp = ctx.enter_context(tc.tile_pool(name="ot", bufs=1))
        ot_t = [otp.tile([128, T], F32R, tag=f"ot{m}", name=f"ot{m}")
                for m in range(NMT)]
        with tc.tile_pool(name="p2pt", bufs=3) as ptp, \
             tc.tile_pool(name="p2n", bufs=1) as nrm, \
             tc.tile_pool(name="p2y", bufs=3) as stgy, \
             tc.tile_pool(name="p2s", bufs=1, space="PSUM") as sbp, \
             tc.tile_pool(name="p2o", bufs=1, space="PSUM") as obp:
            for j in range(NQC):
                nkt = 4 * (j + 1)
                for chains in ((0, 1), (2, 3)):
                    o_ps = {}
                    for c, pr in enumerate(chains):
                        for h in range(2):
                            o_ps[(pr, h)] = obp.tile(
                                [65, 512], F32, tag=f"o{c}_{h}",
                                name=f"o{pr}_{j}_{h}")

                    def emit_pv(pr, pt, kt):
                        for h in range(2):
                            nc.tensor.matmul(
                                o_ps[(pr, h)][:],
                                va_t[kt][:, (2 * pr + h) * 65:
                                         (2 * pr + h) * 65 + 65],
                                pt[:, h * 512:(h + 1) * 512],
                                start=(kt == 0), stop=(kt == nkt - 1))

                    pend = {pr: None for pr in chains}
                    for kt in range(nkt):
                        band = kt - 4 * j >= 0
                        for c, pr in enumerate(chains):
                            sbig = sbp.tile([128, 1024], F32, tag=f"s{c}",
                                            name=f"s{pr}_{j}_{kt}")
                            for h in range(2):
                                pb = 64 * h
                                nc.tensor.matmul(
                                    sbig[:, h * 512:(h + 1) * 512],
                                    kt_t[pr][pb:pb + 64,
                                             kt * 128:(kt + 1) * 128],
                                    qt_t[pr][pb:pb + 64,
                                             j * 512:(j + 1) * 512],
                                    start=True, stop=True,
                                    tile_position=(pb, 0))
                            if band:
                                o = 128 * (kt - 4 * j)
                                for h in range(2):
                                    nc.vector.tensor_add(
                                        sbig[:, h * 512 + o:h * 512 + o + 128],
                                        sbig[:, h * 512 + o:h * 512 + o + 128],
                                        mkt[:])
                            pt = ptp.tile([128, 1024], F32R, tag=f"pt{c}",
                                          name=f"pt{pr}_{j}_{kt}")
                            nc.scalar.activation(pt[:], sbig[:],
                                                 AF.Exp, scale=SCALE)
                            if band and kt - 4 * j > 0:
                                o = 128 * (kt - 4 * j)
                                for h in range(2):
                                    nc.vector.memset(
                                        pt[:, h * 512:h * 512 + o].bitcast(F32),
                                        0.0)
                            if pend[pr] is not None:
                                emit_pv(pr, *pend[pr])
                            pend[pr] = (pt, kt)
                    for pr in chains:
                        emit_pv(pr, *pend[pr])
                    # normalize rows by Z (row 64) and store to OT
                    for c, pr in enumerate(chains):
                        for h in range(2):
                            rr = nrm.tile([1, 512], F32, tag=f"rr{c}{h}",
                                          name=f"rr{pr}_{j}_{h}")
                            nc.vector.reciprocal(rr[:], o_ps[(pr, h)][64:65, :])
                            bc = nrm.tile([64, 512], F32, tag=f"bc{c}{h}",
                                          name=f"bc{pr}_{j}_{h}")
                            nc.gpsimd.partition_broadcast(bc[:], rr[:])
                            nc.vector.tensor_mul(
                                ot_t[pr][h * 64:(h + 1) * 64,
                                         j * 512:(j + 1) * 512],
                                o_ps[(pr, h)][0:64, :], bc[:])
                # out-projection for this q-chunk (reuses O psum banks)
                for ti, tt in enumerate(range(4 * j, 4 * j + 4)):
                    ys = stgy.tile([128, C], F32, tag="ys", name=f"ys{tt}")
                    for ncol in range(2):
                        oc, oh = divmod((ti * 2 + ncol) % 4, 2)
                        ps = obp.tile([128, 512], F32, tag=f"o{oc}_{oh}",
                                      name=f"py{tt}_{ncol}")
                        for pt_i in range(NMT):
                            nc.tensor.matmul(
                                ps[:], ot_t[pt_i][:, tt * 128:(tt + 1) * 128],
                                wo_t[pt_i][:, ncol * 512:(ncol + 1) * 512],
                                start=(pt_i == 0), stop=(pt_i == NMT - 1))
                        nc.scalar.copy(ys[:, ncol * 512:(ncol + 1) * 512],
                                       ps[:])
                    nc.sync.dma_start(yp_d[tt * 128:(tt + 1) * 128, :], ys[:])





def _build_program():
    nc = bacc.Bacc()
    xb_d = nc.dram_tensor("xb", [T, C], F32, kind="ExternalInput")
    wq_d = nc.dram_tensor("wq", [C, GF], F32, kind="ExternalInput")
    wk_d = nc.dram_tensor("wk", [C, GF], F32, kind="ExternalInput")
    wv_d = nc.dram_tensor("wv", [C, GF], F32, kind="ExternalInput")
    wo_d = nc.dram_tensor("wo", [GF, C], F32, kind="ExternalInput")
    bq_d = nc.dram_tensor("bq", [GF, 1], F32, kind="ExternalInput")
    bk_d = nc.dram_tensor("bk", [GF, 1], F32, kind="ExternalInput")
    bva_d = nc.dram_tensor("bva", [1, VW], F32, kind="ExternalInput")
    mk_d = nc.dram_tensor("mk", [128, 128], F32, kind="ExternalInput")
    id_d = nc.dram_tensor("ident", [128, 128], F32, kind="ExternalInput")
    yp_d = nc.dram_tensor("yp", [T, C], F32, kind="ExternalOutput")
    ko_d = nc.dram_tensor("ko", [T, GF], F32, kind="ExternalOutput")
    vo_d = nc.dram_tensor("vo", [T, GF], F32, kind="ExternalOutput")

    with tile.TileContext(nc) as tc:
        _build_body(nc, tc, xb_d, wq_d, wk_d, wv_d, wo_d, bq_d, bk_d, bva_d,
                    mk_d, id_d, yp_d, ko_d, vo_d)
    nc.finalize()
    return nc


def _get_program():
    global _PROGRAM
    if _PROGRAM is None:
        _PROGRAM = _build_program()
    return _PROGRAM


def _make_in_maps(x, W_qkv, b_qkv, W_out, b_out):
    kk = np.arange(128)[:, None]
    qq = np.arange(128)[None, :]
    mk = np.where(kk > qq, np.float32(-1e30), np.float32(0.0))
    ident = np.eye(128, dtype=np.float32)

    in_maps = []
    for c in range(NCORES):
        b, g = divmod(c, 2)
        cs = slice(g * GF, (g + 1) * GF)
        bva = np.zeros((1, VW), dtype=np.float32)
        bv = b_qkv[2 * C + g * GF:2 * C + (g + 1) * GF]
        for h in range(HPC):
            bva[0, h * 65:h * 65 + 64] = bv[h * 64:(h + 1) * 64]
            bva[0, h * 65 + 64] = 1.0
        in_maps.append({
            "xb": np.ascontiguousarray(x[b]),
            "wq": np.ascontiguousarray(W_qkv[:, cs]),
            "wk": np.ascontiguousarray(W_qkv[:, C:][:, cs]),
            "wv": np.ascontiguousarray(W_qkv[:, 2 * C:][:, cs]),
            "wo": np.ascontiguousarray(W_out[cs, :]),
            "bq": np.ascontiguousarray(b_qkv[cs].reshape(GF, 1)),
            "bk": np.ascontiguousarray(b_qkv[C:][cs].reshape(GF, 1)),
            "bva": bva,
            "mk": mk,
            "ident": ident,
        })
    return in_maps


def run(x, W_qkv, b_qkv, W_out, b_out, trace=False):
    x = np.asarray(x, dtype=np.float32)
    W_qkv = np.asarray(W_qkv, dtype=np.float32)
    b_qkv = np.asarray(b_qkv, dtype=np.float32)
    W_out = np.asarray(W_out, dtype=np.float32)
    b_out = np.asarray(b_out, dtype=np.float32)

    nc = _get_program()
    in_maps = _make_in_maps(x, W_qkv, b_qkv, W_out, b_out)
    res = run_bass_kernel_spmd(nc, in_maps, core_ids=list(range(NCORES)),
                               trace=trace)

    y = np.empty((B, T, C), dtype=np.float32)
    k = np.empty((B, NH, T, HD), dtype=np.float32)
    v = np.empty((B, NH, T, HD), dtype=np.float32)
    for b in range(B):
        r0, r1 = res.results[2 * b], res.results[2 * b + 1]
        y[b] = r0["yp"] + r1["yp"] + b_out[None, :]
        for g, r in ((0, r0), (1, r1)):
            k[b, g * HPC:(g + 1) * HPC] = np.moveaxis(
                r["ko"].reshape(T, HPC, HD), 1, 0)
            v[b, g * HPC:(g + 1) * HPC] = np.moveaxis(
                r["vo"].reshape(T, HPC, HD), 1, 0)
    return (y, k, v), res


def kernel(x, W_qkv, b_qkv, W_out, b_out):
    (y, k, v), _ = run(x, W_qkv, b_qkv, W_out, b_out, trace=False)
    return (y, k, v)


# revision 17
# speedup vs baseline: 1.0796x; 1.0210x over previous
"""Causal self-attention kernel for 8 Trainium2 NeuronCores.

Sharding (Megatron-style): core c handles batch b=c//2 and head-group
g=c%2 (8 of 16 heads). qkv projection is column-parallel, out
projection row-parallel; the host sums the two partial y outputs per
batch and adds b_out.

On-device math (per core), all matmuls in float32r (full PE rate):
  xT = x^T                          (PE transpose blocks)
  Q^T = Wq^T xT, K^T = Wk^T xT      (feature-major, [512, 2048])
  V   = x Wv                        (token-major, + ones column per head)
  S^T = K Q^T (per head, row-packed pairs on the PE array)
  P^T = exp(scale*S^T + causal mask)   (no max-subtraction; scores are O(8))
  O^T_unnorm = V_aug^T P^T  (65th row accumulates the softmax denominator Z)
  O^T = O^T_unnorm * broadcast(1/Z)
  y_partial = O^T^T W_out_slice     (token-major)

Attention runs two independent (head-pair, q-chunk) chains interleaved
so ScalarE (exp) and the PE pipeline each other without ping-pong.
"""

import sys

sys.path.insert(0, "/opt/trn_rl_repo")

import numpy as np

from concourse import bacc
import concourse.mybir as mybir
import concourse.tile as tile
from concourse.bass_utils import run_bass_kernel_spmd

F32 = mybir.dt.float32
F32R = mybir.dt.float32r
AF = mybir.ActivationFunctionType

B, T, C = 4, 2048, 1024
NH, HD = 16, 64
HPC = 8          # heads per core
GF = HPC * HD    # 512: per-core q/k/v feature width
NCORES = 8
SCALE = 1.0 / np.sqrt(HD)

PT128 = T // 128   # 16 token tiles
NQC = T // 512     # 4 q-chunks
KT_C = C // 128    # 8 contraction tiles for x projections
NMT = GF // 128    # 4 feature tiles (2 heads each)
VW = HPC * (HD + 1)  # 520: V_aug width

_PROGRAM = None


def _build_body(nc, tc, xb_d, wq_d, wk_d, wv_d, wo_d, bq_d, bk_d, bva_d,
                mk_d, id_d, yp_d, ko_d, vo_d):
    from contextlib import ExitStack
    with ExitStack() as ctx:
        cst = ctx.enter_context(tc.tile_pool(name="cst", bufs=1))
        qtkt = ctx.enter_context(tc.tile_pool(name="qtkt", bufs=1))
        vap = ctx.enter_context(tc.tile_pool(name="vaug", bufs=1))

        # ---- constants ----
        ident = cst.tile([128, 128], F32, tag="ident")
        nc.sync.dma_start(ident[:], id_d[:])
        identr = cst.tile([128, 128], F32R, tag="identr")
        nc.vector.tensor_copy(identr[:], ident[:])
        mkt = cst.tile([128, 128], F32, tag="mkt")
        nc.sync.dma_start(mkt[:], mk_d[:])
        bq_t, bk_t = [], []
        for m in range(NMT):
            t1 = cst.tile([128, 1], F32, tag=f"bq{m}", name=f"bq{m}")
            nc.sync.dma_start(t1[:], bq_d[m * 128:(m + 1) * 128, :])
            bq_t.append(t1)
            t2 = cst.tile([128, 1], F32, tag=f"bk{m}", name=f"bk{m}")
            nc.sync.dma_start(t2[:], bk_d[m * 128:(m + 1) * 128, :])
            bk_t.append(t2)
        bva_row = cst.tile([1, VW], F32, tag="bvarow")
        nc.sync.dma_start(bva_row[:], bva_d[:])
        bvb = cst.tile([128, VW], F32, tag="bvb")
        nc.gpsimd.partition_broadcast(bvb[:], bva_row[:])

        qt_t = [qtkt.tile([128, T], F32R, tag=f"qt{m}", name=f"qt{m}")
                for m in range(NMT)]
        kt_t = [qtkt.tile([128, T], F32R, tag=f"kt{m}", name=f"kt{m}")
                for m in range(NMT)]
        va_t = [vap.tile([128, VW], F32R, tag=f"va{i}", name=f"va{i}")
                for i in range(PT128)]

        # ================= Phase 1: projections (two token halves) ========
        for half in range(2):
            t0 = half * (T // 2)
            ntt = T // 2 // 128
            with tc.tile_pool(name=f"p1s{half}", bufs=4) as stg, \
                 tc.tile_pool(name=f"p1x{half}", bufs=1) as xtp, \
                 tc.tile_pool(name=f"p1w{half}", bufs=1) as wp, \
                 tc.tile_pool(name=f"p1pa{half}", bufs=3,
                              space="PSUM") as psa, \
                 tc.tile_pool(name=f"p1pb{half}", bufs=4,
                              space="PSUM") as psb:
                # prefetch Wq for this half (cast on Pool, during x DMA)
                wq_t = []
                for kt in range(KT_C):
                    ws = stg.tile([128, GF], F32, tag="ws",
                                  name=f"wspre{half}_{kt}")
                    nc.sync.dma_start(ws[:], wq_d[kt * 128:(kt + 1) * 128, :])
                    wr = wp.tile([128, GF], F32R, tag=f"w{kt}",
                                 name=f"wpre{half}_{kt}")
                    nc.gpsimd.tensor_copy(wr[:], ws[:])
                    wq_t.append(wr)

                # ---- x^T for this half ----
                xt_t = [xtp.tile([128, T // 2], F32R, tag=f"xt{fb}",
                                 name=f"xt{half}_{fb}")
                        for fb in range(KT_C)]
                for tt in range(ntt):
                    xs = stg.tile([128, C], F32, tag="xs",
                                  name=f"xs{half}_{tt}")
                    nc.scalar.dma_start(
                        xs[:], xb_d[t0 + tt * 128:t0 + (tt + 1) * 128, :])
                    for fb in range(KT_C):
                        tp = psa.tile([128, 512], F32, tag="tp",
                                      name=f"tp{half}_{tt}_{fb}")
                        nc.tensor.transpose(
                            tp[:, 0:128], xs[:, fb * 128:(fb + 1) * 128],
                            ident[:])
                        nc.vector.tensor_copy(
                            xt_t[fb][:, tt * 128:(tt + 1) * 128],
                            tp[:, 0:128])

                # ---- Q^T and K^T (feature-major) ----
                for which, w_d, bias_t, dst in (
                        ("q", wq_d, bq_t, qt_t), ("k", wk_d, bk_t, kt_t)):
                    if which == "q":
                        w_t = wq_t
                    else:
                        w_t = []
                        for kt in range(KT_C):
                            ws = stg.tile([128, GF], F32, tag="ws",
                                          name=f"ws{half}k_{kt}")
                            nc.sync.dma_start(
                                ws[:], w_d[kt * 128:(kt + 1) * 128, :])
                            wr = wp.tile([128, GF], F32R, tag=f"w{kt}",
                                         name=f"w{half}k_{kt}")
                            nc.gpsimd.tensor_copy(wr[:], ws[:])
                            w_t.append(wr)
                    for m in range(NMT):
                        for n in range(T // 2 // 512):
                            ps = psb.tile([128, 512], F32, tag="pj",
                                          name=f"pj{half}{which}_{m}_{n}")
                            for kt in range(KT_C):
                                nc.tensor.matmul(
                                    ps[:],
                                    w_t[kt][:, m * 128:(m + 1) * 128],
                                    xt_t[kt][:, n * 512:(n + 1) * 512],
                                    start=(kt == 0), stop=(kt == KT_C - 1))
                            nc.vector.tensor_scalar_add(
                                dst[m][:, t0 + n * 512:t0 + (n + 1) * 512],
                                ps[:], bias_t[m][:])

                # ---- V (token-major) + V_aug + v output ----
                w_t = []
                for kt in range(KT_C):
                    ws = stg.tile([128, GF], F32, tag="ws",
                                  name=f"ws{half}v_{kt}")
                    nc.sync.dma_start(ws[:], wv_d[kt * 128:(kt + 1) * 128, :])
                    wr = wp.tile([128, GF], F32R, tag=f"w{kt}",
                                 name=f"w{half}v_{kt}")
                    nc.gpsimd.tensor_copy(wr[:], ws[:])
                    w_t.append(wr)
                for tt in range(ntt):
                    gt = half * ntt + tt
                    ps = psb.tile([128, GF], F32, tag="pj",
                                  name=f"pjv{half}_{tt}")
                    for kt in range(KT_C):
                        nc.tensor.matmul(
                            ps[:], xt_t[kt][:, tt * 128:(tt + 1) * 128],
                            w_t[kt][:], start=(kt == 0),
                            stop=(kt == KT_C - 1))
                    vs = stg.tile([128, GF], F32, tag="vs",
                                  name=f"vs{half}_{tt}")
                    nc.scalar.copy(vs[:], ps[:])
                    nc.scalar.dma_start(
                        vo_d[t0 + tt * 128:t0 + (tt + 1) * 128, :], vs[:])
                    for h in range(HPC):
                        nc.vector.tensor_add(
                            va_t[gt][:, h * 65:h * 65 + 64],
                            ps[:, h * 64:(h + 1) * 64],
                            bvb[:, h * 65:h * 65 + 64])
                    nc.gpsimd.tensor_copy(
                        va_t[gt][:].rearrange(
                            "p (h c) -> p h c", c=65)[:, :, 64:65],
                        bvb[:].rearrange(
                            "p (h c) -> p h c", c=65)[:, :, 64:65])

                # ---- k output: transpose K^T back to token-major ----
                for tt in range(ntt):
                    ks = stg.tile([128, GF], F32, tag="ks",
                                  name=f"ks{half}_{tt}")
                    for m in range(NMT):
                        tp = psa.tile([128, 512], F32, tag="tp",
                                      name=f"ktp{half}_{m}_{tt}")
                        nc.tensor.transpose(
                            tp[:, 0:128].bitcast(F32R),
                            kt_t[m][:, t0 + tt * 128:t0 + (tt + 1) * 128],
                            identr[:])
                        nc.vector.tensor_copy(
                            ks[:, m * 128:(m + 1) * 128],
                            tp[:, 0:128].bitcast(F32R))
                    nc.scalar.dma_start(
                        ko_d[t0 + tt * 128:t0 + (tt + 1) * 128, :], ks[:])


        # ---- prefetch + cast W_out early (Pool/DMA idle in P2) ----
        wp3 = ctx.enter_context(tc.tile_pool(name="p3w", bufs=1))
        wo_t = []
        with tc.tile_pool(name="wostg", bufs=2) as wostg:
            for pt_i in range(NMT):
                ws = wostg.tile([128, C], F32, tag="ws3", name=f"wo_s{pt_i}")
                nc.sync.dma_start(ws[:], wo_d[pt_i * 128:(pt_i + 1) * 128, :])
                wr = wp3.tile([128, C], F32R, tag=f"wo{pt_i}",
                              name=f"wo_r{pt_i}")
                nc.gpsimd.tensor_copy(wr[:], ws[:])
                wo_t.append(wr)

        # ================= Phase 2: attention =============================
        otp = ctx.enter_context(tc.tile_pool(name="ot", bufs=1))
        ot_t = [otp.tile([128, T], F32R, tag=f"ot{m}", name=f"ot{m}")
                for m in range(NMT)]
        with tc.tile_pool(name="p2pt", bufs=3) as ptp, \
             tc.tile_pool(name="p2n", bufs=1) as nrm, \
             tc.tile_pool(name="p2y", bufs=3) as stgy, \
             tc.tile_pool(name="p2s", bufs=1, space="PSUM") as sbp, \
             tc.tile_pool(name="p2o", bufs=1, space="PSUM") as obp:
            for j in range(NQC):
                nkt = 4 * (j + 1)
                for chains in ((0, 1), (2, 3)):
                    o_ps = {}
                    for c, pr in enumerate(chains):
                        for h in range(2):
                            o_ps[(pr, h)] = obp.tile(
                                [65, 512], F32, tag=f"o{c}_{h}",
                                name=f"o{pr}_{j}_{h}")

                    def emit_pv(pr, pt, kt):
                        for h in range(2):
                            nc.tensor.matmul(
                                o_ps[(pr, h)][:],
                                va_t[kt][:, (2 * pr + h) * 65:
                                         (2 * pr + h) * 65 + 65],
                                pt[:, h * 512:(h + 1) * 512],
                                start=(kt == 0), stop=(kt == nkt - 1))

                    pend = {pr: None for pr in chains}
                    for kt in range(nkt):
                        band = kt - 4 * j >= 0
                        for c, pr in enumerate(chains):
                            sbig = sbp.tile([128, 1024], F32, tag=f"s{c}",
                                            name=f"s{pr}_{j}_{kt}")
                            for h in range(2):
                                pb = 64 * h
                                nc.tensor.matmul(
                                    sbig[:, h * 512:(h + 1) * 512],
                                    kt_t[pr][pb:pb + 64,
                                             kt * 128:(kt + 1) * 128],
                                    qt_t[pr][pb:pb + 64,
                                             j * 512:(j + 1) * 512],
                                    start=True, stop=True,
                                    tile_position=(pb, 0))
                            if band:
                                o = 128 * (kt - 4 * j)
                                for h in range(2):
                                    nc.vector.tensor_add(
                                        sbig[:, h * 512 + o:h * 512 + o + 128],
                                        sbig[:, h * 512 + o:h * 512 + o + 128],
                                        mkt[:])
                            pt = ptp.tile([128, 1024], F32R, tag=f"pt{c}",
                                          name=f"pt{pr}_{j}_{kt}")
                            nc.scalar.activation(pt[:], sbig[:],
                                                 AF.Exp, scale=SCALE)
                            if band and kt - 4 * j > 0:
                                o = 128 * (kt - 4 * j)
                                for h in range(2):
                                    nc.vector.memset(
                                        pt[:, h * 512:h * 512 + o].bitcast(F32),
                                        0.0)
                            if pend[pr] is not None:
                                emit_pv(pr, *pend[pr])
                            pend[pr] = (pt, kt)
                    for pr in chains:
                        emit_pv(pr, *pend[pr])
                    # normalize rows by Z (row 64) and store to OT
                    for c, pr in enumerate(chains):
                        for h in range(2):
                            rr = nrm.tile([1, 512], F32, tag=f"rr{c}{h}",
                                          name=f"rr{pr}_{j}_{h}")
                            nc.vector.reciprocal(rr[:], o_ps[(pr, h)][64:65, :])
                            bc = nrm.tile([64, 512], F32, tag=f"bc{c}{h}",
                                          name=f"bc{pr}_{j}_{h}")
                            nc.gpsimd.partition_broadcast(bc[:], rr[:])
                            nc.vector.tensor_mul(
                                ot_t[pr][h * 64:(h + 1) * 64,
                                         j * 512:(j + 1) * 512],
                                o_ps[(pr, h)][0:64, :], bc[:])
                # out-projection for this q-chunk (reuses O psum banks)
                for ti, tt in enumerate(range(4 * j, 4 * j + 4)):
                    ys = stgy.tile([128, C], F32, tag="ys", name=f"ys{tt}")
                    for ncol in range(2):
                        oc, oh = divmod((ti * 2 + ncol) % 4, 2)
                        ps = obp.tile([128, 512], F32, tag=f"o{oc}_{oh}",
                                      name=f"py{tt}_{ncol}")
                        for pt_i in range(NMT):
                            nc.tensor.matmul(
                                ps[:], ot_t[pt_i][:, tt * 128:(tt + 1) * 128],
                                wo_t[pt_i][:, ncol * 512:(ncol + 1) * 512],
                                start=(pt_i == 0), stop=(pt_i == NMT - 1))
                        nc.scalar.copy(ys[:, ncol * 512:(ncol + 1) * 512],
                                       ps[:])
                    nc.sync.dma_start(yp_d[tt * 128:(tt + 1) * 128, :], ys[:])





def _build_program():
    nc = bacc.Bacc()
    xb_d = nc.dram_tensor("xb", [T, C], F32, kind="ExternalInput")
    wq_d = nc.dram_tensor("wq", [C, GF], F32, kind="ExternalInput")
    wk_d = nc.dram_tensor("wk", [C, GF], F32, kind="ExternalInput")
    wv_d = nc.dram_tensor("wv", [C, GF], F32, kind="ExternalInput")
    wo_d = nc.dram_tensor("wo", [GF, C], F32, kind="ExternalInput")
    bq_d = nc.dram_tensor("bq", [GF, 1], F32, kind="ExternalInput")
    bk_d = nc.dram_tensor("bk", [GF, 1], F32, kind="ExternalInput")
    bva_d = nc.dram_tensor("bva", [1, VW], F32, kind="ExternalInput")
    mk_d = nc.dram_tensor("mk", [128, 128], F32, kind="ExternalInput")
    id_d = nc.dram_tensor("ident", [128, 128], F32, kind="ExternalInput")
    yp_d = nc.dram_tensor("yp", [T, C], F32, kind="ExternalOutput")
    ko_d = nc.dram_tensor("ko", [T, GF], F32, kind="ExternalOutput")
    vo_d = nc.dram_tensor("vo", [T, GF], F32, kind="ExternalOutput")

    with tile.TileContext(nc) as tc:
        _build_body(nc, tc, xb_d, wq_d, wk_d, wv_d, wo_d, bq_d, bk_d, bva_d,
                    mk_d, id_d, yp_d, ko_d, vo_d)
    nc.finalize()
    return nc


def _get_program():
    global _PROGRAM
    if _PROGRAM is None:
        _PROGRAM = _build_program()
    return _PROGRAM


def _make_in_maps(x, W_qkv, b_qkv, W_out, b_out):
    kk = np.arange(128)[:, None]
    qq = np.arange(128)[None, :]
    mk = np.where(kk > qq, np.float32(-1e30), np.float32(0.0))
    ident = np.eye(128, dtype=np.float32)

    in_maps = []
    for c in range(NCORES):
        b, g = divmod(c, 2)
        cs = slice(g * GF, (g + 1) * GF)
        bva = np.zeros((1, VW), dtype=np.float32)
        bv = b_qkv[2 * C + g * GF:2 * C + (g + 1) * GF]
        for h in range(HPC):
            bva[0, h * 65:h * 65 + 64] = bv[h * 64:(h + 1) * 64]
            bva[0, h * 65 + 64] = 1.0
        in_maps.append({
            "xb": np.ascontiguousarray(x[b]),
            "wq": np.ascontiguousarray(W_qkv[:, cs]),
            "wk": np.ascontiguousarray(W_qkv[:, C:][:, cs]),
            "wv": np.ascontiguousarray(W_qkv[:, 2 * C:][:, cs]),
            "wo": np.ascontiguousarray(W_out[cs, :]),
            "bq": np.ascontiguousarray(b_qkv[cs].reshape(GF, 1)),
            "bk": np.ascontiguousarray(b_qkv[C:][cs].reshape(GF, 1)),
            "bva": bva,
            "mk": mk,
            "ident": ident,
        })
    return in_maps


def run(x, W_qkv, b_qkv, W_out, b_out, trace=False):
    x = np.asarray(x, dtype=np.float32)
    W_qkv = np.asarray(W_qkv, dtype=np.float32)
    b_qkv = np.asarray(b_qkv, dtype=np.float32)
    W_out = np.asarray(W_out, dtype=np.float32)
    b_out = np.asarray(b_out, dtype=np.float32)

    nc = _get_program()
    in_maps = _make_in_maps(x, W_qkv, b_qkv, W_out, b_out)
    res = run_bass_kernel_spmd(nc, in_maps, core_ids=list(range(NCORES)),
                               trace=trace)

    y = np.empty((B, T, C), dtype=np.float32)
    k = np.empty((B, NH, T, HD), dtype=np.float32)
    v = np.empty((B, NH, T, HD), dtype=np.float32)
    for b in range(B):
        r0, r1 = res.results[2 * b], res.results[2 * b + 1]
        y[b] = r0["yp"] + r1["yp"] + b_out[None, :]
        for g, r in ((0, r0), (1, r1)):
            k[b, g * HPC:(g + 1) * HPC] = np.moveaxis(
                r["ko"].reshape(T, HPC, HD), 1, 0)
            v[b, g * HPC:(g + 1) * HPC] = np.moveaxis(
                r["vo"].reshape(T, HPC, HD), 1, 0)
    return (y, k, v), res


def kernel(x, W_qkv, b_qkv, W_out, b_out):
    (y, k, v), _ = run(x, W_qkv, b_qkv, W_out, b_out, trace=False)
    return (y, k, v)


# revision 18
# speedup vs baseline: 1.0855x; 1.0055x over previous
"""Causal self-attention kernel for 8 Trainium2 NeuronCores.

Sharding (Megatron-style): core c handles batch b=c//2 and head-group
g=c%2 (8 of 16 heads). qkv projection is column-parallel, out
projection row-parallel; the host sums the two partial y outputs per
batch and adds b_out.

On-device math (per core), all matmuls in float32r (full PE rate):
  xT = x^T                          (PE transpose blocks)
  Q^T = Wq^T xT, K^T = Wk^T xT      (feature-major, [512, 2048])
  V   = x Wv                        (token-major, + ones column per head)
  S^T = K Q^T (per head, row-packed pairs on the PE array)
  P^T = exp(scale*S^T + causal mask)   (no max-subtraction; scores are O(8))
  O^T_unnorm = V_aug^T P^T  (65th row accumulates the softmax denominator Z)
  O^T = O^T_unnorm * broadcast(1/Z)
  y_partial = O^T^T W_out_slice     (token-major)

Attention runs two independent (head-pair, q-chunk) chains interleaved
so ScalarE (exp) and the PE pipeline each other without ping-pong.
"""

import sys

sys.path.insert(0, "/opt/trn_rl_repo")

import numpy as np

from concourse import bacc
import concourse.mybir as mybir
import concourse.tile as tile
from concourse.bass_utils import run_bass_kernel_spmd

F32 = mybir.dt.float32
F32R = mybir.dt.float32r
AF = mybir.ActivationFunctionType

B, T, C = 4, 2048, 1024
NH, HD = 16, 64
HPC = 8          # heads per core
GF = HPC * HD    # 512: per-core q/k/v feature width
NCORES = 8
SCALE = 1.0 / np.sqrt(HD)

PT128 = T // 128   # 16 token tiles
NQC = T // 512     # 4 q-chunks
KT_C = C // 128    # 8 contraction tiles for x projections
NMT = GF // 128    # 4 feature tiles (2 heads each)
VW = HPC * (HD + 1)  # 520: V_aug width

_PROGRAM = None


def _build_body(nc, tc, xb_d, wq_d, wk_d, wv_d, wo_d, bq_d, bk_d, bva_d,
                mk_d, id_d, yp_d, ko_d, vo_d):
    from contextlib import ExitStack
    with ExitStack() as ctx:
        cst = ctx.enter_context(tc.tile_pool(name="cst", bufs=1))
        qtkt = ctx.enter_context(tc.tile_pool(name="qtkt", bufs=1))
        vap = ctx.enter_context(tc.tile_pool(name="vaug", bufs=1))

        # ---- constants ----
        ident = cst.tile([128, 128], F32, tag="ident")
        nc.sync.dma_start(ident[:], id_d[:])
        identr = cst.tile([128, 128], F32R, tag="identr")
        nc.vector.tensor_copy(identr[:], ident[:])
        mkt = cst.tile([128, 128], F32, tag="mkt")
        nc.sync.dma_start(mkt[:], mk_d[:])
        bq_t, bk_t = [], []
        for m in range(NMT):
            t1 = cst.tile([128, 1], F32, tag=f"bq{m}", name=f"bq{m}")
            nc.sync.dma_start(t1[:], bq_d[m * 128:(m + 1) * 128, :])
            bq_t.append(t1)
            t2 = cst.tile([128, 1], F32, tag=f"bk{m}", name=f"bk{m}")
            nc.sync.dma_start(t2[:], bk_d[m * 128:(m + 1) * 128, :])
            bk_t.append(t2)
        bva_row = cst.tile([1, VW], F32, tag="bvarow")
        nc.sync.dma_start(bva_row[:], bva_d[:])
        bvb = cst.tile([128, VW], F32, tag="bvb")
        nc.gpsimd.partition_broadcast(bvb[:], bva_row[:])

        qt_t = [qtkt.tile([128, T], F32R, tag=f"qt{m}", name=f"qt{m}")
                for m in range(NMT)]
        kt_t = [qtkt.tile([128, T], F32R, tag=f"kt{m}", name=f"kt{m}")
                for m in range(NMT)]
        va_t = [vap.tile([128, VW], F32R, tag=f"va{i}", name=f"va{i}")
                for i in range(PT128)]

        # ================= Phase 1: projections (two token halves) ========
        for half in range(2):
            t0 = half * (T // 2)
            ntt = T // 2 // 128
            with tc.tile_pool(name=f"p1s{half}", bufs=4) as stg, \
                 tc.tile_pool(name=f"p1x{half}", bufs=1) as xtp, \
                 tc.tile_pool(name=f"p1w{half}", bufs=1) as wp, \
                 tc.tile_pool(name=f"p1pa{half}", bufs=4,
                              space="PSUM") as psa, \
                 tc.tile_pool(name=f"p1pb{half}", bufs=4,
                              space="PSUM") as psb:
                # prefetch Wq for this half (cast on Pool, during x DMA)
                wq_t = []
                for kt in range(KT_C):
                    ws = stg.tile([128, GF], F32, tag="ws",
                                  name=f"wspre{half}_{kt}")
                    nc.sync.dma_start(ws[:], wq_d[kt * 128:(kt + 1) * 128, :])
                    wr = wp.tile([128, GF], F32R, tag=f"w{kt}",
                                 name=f"wpre{half}_{kt}")
                    nc.gpsimd.tensor_copy(wr[:], ws[:])
                    wq_t.append(wr)

                # ---- x^T for this half ----
                xt_t = [xtp.tile([128, T // 2], F32R, tag=f"xt{fb}",
                                 name=f"xt{half}_{fb}")
                        for fb in range(KT_C)]
                for tt in range(ntt):
                    xs = stg.tile([128, C], F32, tag="xs",
                                  name=f"xs{half}_{tt}")
                    nc.scalar.dma_start(
                        xs[:], xb_d[t0 + tt * 128:t0 + (tt + 1) * 128, :])
                    for fb in range(KT_C):
                        tp = psa.tile([128, 512], F32, tag="tp",
                                      name=f"tp{half}_{tt}_{fb}")
                        nc.tensor.transpose(
                            tp[:, 0:128], xs[:, fb * 128:(fb + 1) * 128],
                            ident[:])
                        nc.vector.tensor_copy(
                            xt_t[fb][:, tt * 128:(tt + 1) * 128],
                            tp[:, 0:128])

                # ---- Q^T and K^T (feature-major) ----
                for which, w_d, bias_t, dst in (
                        ("q", wq_d, bq_t, qt_t), ("k", wk_d, bk_t, kt_t)):
                    if which == "q":
                        w_t = wq_t
                    else:
                        w_t = []
                        for kt in range(KT_C):
                            ws = stg.tile([128, GF], F32, tag="ws",
                                          name=f"ws{half}k_{kt}")
                            nc.sync.dma_start(
                                ws[:], w_d[kt * 128:(kt + 1) * 128, :])
                            wr = wp.tile([128, GF], F32R, tag=f"w{kt}",
                                         name=f"w{half}k_{kt}")
                            nc.gpsimd.tensor_copy(wr[:], ws[:])
                            w_t.append(wr)
                    for m in range(NMT):
                        for n in range(T // 2 // 512):
                            ps = psb.tile([128, 512], F32, tag="pj",
                                          name=f"pj{half}{which}_{m}_{n}")
                            for kt in range(KT_C):
                                nc.tensor.matmul(
                                    ps[:],
                                    w_t[kt][:, m * 128:(m + 1) * 128],
                                    xt_t[kt][:, n * 512:(n + 1) * 512],
                                    start=(kt == 0), stop=(kt == KT_C - 1))
                            nc.vector.tensor_scalar_add(
                                dst[m][:, t0 + n * 512:t0 + (n + 1) * 512],
                                ps[:], bias_t[m][:])

                # ---- V (token-major) + V_aug + v output ----
                w_t = []
                for kt in range(KT_C):
                    ws = stg.tile([128, GF], F32, tag="ws",
                                  name=f"ws{half}v_{kt}")
                    nc.sync.dma_start(ws[:], wv_d[kt * 128:(kt + 1) * 128, :])
                    wr = wp.tile([128, GF], F32R, tag=f"w{kt}",
                                 name=f"w{half}v_{kt}")
                    nc.gpsimd.tensor_copy(wr[:], ws[:])
                    w_t.append(wr)
                for tt in range(ntt):
                    gt = half * ntt + tt
                    ps = psb.tile([128, GF], F32, tag="pj",
                                  name=f"pjv{half}_{tt}")
                    for kt in range(KT_C):
                        nc.tensor.matmul(
                            ps[:], xt_t[kt][:, tt * 128:(tt + 1) * 128],
                            w_t[kt][:], start=(kt == 0),
                            stop=(kt == KT_C - 1))
                    vs = stg.tile([128, GF], F32, tag="vs",
                                  name=f"vs{half}_{tt}")
                    nc.scalar.copy(vs[:], ps[:])
                    nc.scalar.dma_start(
                        vo_d[t0 + tt * 128:t0 + (tt + 1) * 128, :], vs[:])
                    for h in range(HPC):
                        nc.vector.tensor_add(
                            va_t[gt][:, h * 65:h * 65 + 64],
                            ps[:, h * 64:(h + 1) * 64],
                            bvb[:, h * 65:h * 65 + 64])
                    nc.gpsimd.tensor_copy(
                        va_t[gt][:].rearrange(
                            "p (h c) -> p h c", c=65)[:, :, 64:65],
                        bvb[:].rearrange(
                            "p (h c) -> p h c", c=65)[:, :, 64:65])

                # ---- k output: transpose K^T back to token-major ----
                for tt in range(ntt):
                    ks = stg.tile([128, GF], F32, tag="ks",
                                  name=f"ks{half}_{tt}")
                    for m in range(NMT):
                        tp = psa.tile([128, 512], F32, tag="tp",
                                      name=f"ktp{half}_{m}_{tt}")
                        nc.tensor.transpose(
                            tp[:, 0:128].bitcast(F32R),
                            kt_t[m][:, t0 + tt * 128:t0 + (tt + 1) * 128],
                            identr[:])
                        nc.vector.tensor_copy(
                            ks[:, m * 128:(m + 1) * 128],
                            tp[:, 0:128].bitcast(F32R))
                    nc.scalar.dma_start(
                        ko_d[t0 + tt * 128:t0 + (tt + 1) * 128, :], ks[:])


        # ---- prefetch + cast W_out early (Pool/DMA idle in P2) ----
        wp3 = ctx.enter_context(tc.tile_pool(name="p3w", bufs=1))
        wo_t = []
        with tc.tile_pool(name="wostg", bufs=3) as wostg:
            for pt_i in range(NMT):
                ws = wostg.tile([128, C], F32, tag="ws3", name=f"wo_s{pt_i}")
                nc.sync.dma_start(ws[:], wo_d[pt_i * 128:(pt_i + 1) * 128, :])
                wr = wp3.tile([128, C], F32R, tag=f"wo{pt_i}",
                              name=f"wo_r{pt_i}")
                nc.gpsimd.tensor_copy(wr[:], ws[:])
                wo_t.append(wr)

        # ================= Phase 2: attention =============================
        otp = ctx.enter_context(tc.tile_pool(name="ot", bufs=1))
        ot_t = [otp.tile([128, T], F32R, tag=f"ot{m}", name=f"ot{m}")
                for m in range(NMT)]
        with tc.tile_pool(name="p2pt", bufs=3) as ptp, \
             tc.tile_pool(name="p2n", bufs=1) as nrm, \
             tc.tile_pool(name="p2y", bufs=3) as stgy, \
             tc.tile_pool(name="p2s", bufs=1, space="PSUM") as sbp, \
             tc.tile_pool(name="p2o", bufs=1, space="PSUM") as obp:
            for j in range(NQC):
                nkt = 4 * (j + 1)
                for chains in ((0, 1), (2, 3)):
                    o_ps = {}
                    for c, pr in enumerate(chains):
                        for h in range(2):
                            o_ps[(pr, h)] = obp.tile(
                                [65, 512], F32, tag=f"o{c}_{h}",
                                name=f"o{pr}_{j}_{h}")

                    def emit_pv(pr, pt, kt):
                        for h in range(2):
                            nc.tensor.matmul(
                                o_ps[(pr, h)][:],
                                va_t[kt][:, (2 * pr + h) * 65:
                                         (2 * pr + h) * 65 + 65],
                                pt[:, h * 512:(h + 1) * 512],
                                start=(kt == 0), stop=(kt == nkt - 1))

                    pend = {pr: None for pr in chains}
                    for kt in range(nkt):
                        band = kt - 4 * j >= 0
                        for c, pr in enumerate(chains):
                            sbig = sbp.tile([128, 1024], F32, tag=f"s{c}",
                                            name=f"s{pr}_{j}_{kt}")
                            for h in range(2):
                                pb = 64 * h
                                nc.tensor.matmul(
                                    sbig[:, h * 512:(h + 1) * 512],
                                    kt_t[pr][pb:pb + 64,
                                             kt * 128:(kt + 1) * 128],
                                    qt_t[pr][pb:pb + 64,
                                             j * 512:(j + 1) * 512],
                                    start=True, stop=True,
                                    tile_position=(pb, 0))
                            if band:
                                o = 128 * (kt - 4 * j)
                                for h in range(2):
                                    nc.vector.tensor_add(
                                        sbig[:, h * 512 + o:h * 512 + o + 128],
                                        sbig[:, h * 512 + o:h * 512 + o + 128],
                                        mkt[:])
                            pt = ptp.tile([128, 1024], F32R, tag=f"pt{c}",
                                          name=f"pt{pr}_{j}_{kt}")
                            nc.scalar.activation(pt[:], sbig[:],
                                                 AF.Exp, scale=SCALE)
                            if band and kt - 4 * j > 0:
                                o = 128 * (kt - 4 * j)
                                for h in range(2):
                                    nc.vector.memset(
                                        pt[:, h * 512:h * 512 + o].bitcast(F32),
                                        0.0)
                            if pend[pr] is not None:
                                emit_pv(pr, *pend[pr])
                            pend[pr] = (pt, kt)
                    for pr in chains:
                        emit_pv(pr, *pend[pr])
                    # normalize rows by Z (row 64) and store to OT
                    for c, pr in enumerate(chains):
                        for h in range(2):
                            rr = nrm.tile([1, 512], F32, tag=f"rr{c}{h}",
                                          name=f"rr{pr}_{j}_{h}")
                            nc.vector.reciprocal(rr[:], o_ps[(pr, h)][64:65, :])
                            bc = nrm.tile([64, 512], F32, tag=f"bc{c}{h}",
                                          name=f"bc{pr}_{j}_{h}")
                            nc.gpsimd.partition_broadcast(bc[:], rr[:])
                            nc.vector.tensor_mul(
                                ot_t[pr][h * 64:(h + 1) * 64,
                                         j * 512:(j + 1) * 512],
                                o_ps[(pr, h)][0:64, :], bc[:])
                # out-projection for this q-chunk (reuses O psum banks)
                for ti, tt in enumerate(range(4 * j, 4 * j + 4)):
                    ys = stgy.tile([128, C], F32, tag="ys", name=f"ys{tt}")
                    for ncol in range(2):
                        oc, oh = divmod((ti * 2 + ncol) % 4, 2)
                        ps = obp.tile([128, 512], F32, tag=f"o{oc}_{oh}",
                                      name=f"py{tt}_{ncol}")
                        for pt_i in range(NMT):
                            nc.tensor.matmul(
                                ps[:], ot_t[pt_i][:, tt * 128:(tt + 1) * 128],
                                wo_t[pt_i][:, ncol * 512:(ncol + 1) * 512],
                                start=(pt_i == 0), stop=(pt_i == NMT - 1))
                        nc.scalar.copy(ys[:, ncol * 512:(ncol + 1) * 512],
                                       ps[:])
                    nc.sync.dma_start(yp_d[tt * 128:(tt + 1) * 128, :], ys[:])





def _build_program():
    nc = bacc.Bacc()
    xb_d = nc.dram_tensor("xb", [T, C], F32, kind="ExternalInput")
    wq_d = nc.dram_tensor("wq", [C, GF], F32, kind="ExternalInput")
    wk_d = nc.dram_tensor("wk", [C, GF], F32, kind="ExternalInput")
    wv_d = nc.dram_tensor("wv", [C, GF], F32, kind="ExternalInput")
    wo_d = nc.dram_tensor("wo", [GF, C], F32, kind="ExternalInput")
    bq_d = nc.dram_tensor("bq", [GF, 1], F32, kind="ExternalInput")
    bk_d = nc.dram_tensor("bk", [GF, 1], F32, kind="ExternalInput")
    bva_d = nc.dram_tensor("bva", [1, VW], F32, kind="ExternalInput")
    mk_d = nc.dram_tensor("mk", [128, 128], F32, kind="ExternalInput")
    id_d = nc.dram_tensor("ident", [128, 128], F32, kind="ExternalInput")
    yp_d = nc.dram_tensor("yp", [T, C], F32, kind="ExternalOutput")
    ko_d = nc.dram_tensor("ko", [T, GF], F32, kind="ExternalOutput")
    vo_d = nc.dram_tensor("vo", [T, GF], F32, kind="ExternalOutput")

    with tile.TileContext(nc) as tc:
        _build_body(nc, tc, xb_d, wq_d, wk_d, wv_d, wo_d, bq_d, bk_d, bva_d,
                    mk_d, id_d, yp_d, ko_d, vo_d)
    nc.finalize()
    return nc


def _get_program():
    global _PROGRAM
    if _PROGRAM is None:
        _PROGRAM = _build_program()
    return _PROGRAM


def _make_in_maps(x, W_qkv, b_qkv, W_out, b_out):
    kk = np.arange(128)[:, None]
    qq = np.arange(128)[None, :]
    mk = np.where(kk > qq, np.float32(-1e30), np.float32(0.0))
    ident = np.eye(128, dtype=np.float32)

    in_maps = []
    for c in range(NCORES):
        b, g = divmod(c, 2)
        cs = slice(g * GF, (g + 1) * GF)
        bva = np.zeros((1, VW), dtype=np.float32)
        bv = b_qkv[2 * C + g * GF:2 * C + (g + 1) * GF]
        for h in range(HPC):
            bva[0, h * 65:h * 65 + 64] = bv[h * 64:(h + 1) * 64]
            bva[0, h * 65 + 64] = 1.0
        in_maps.append({
            "xb": np.ascontiguousarray(x[b]),
            "wq": np.ascontiguousarray(W_qkv[:, cs]),
            "wk": np.ascontiguousarray(W_qkv[:, C:][:, cs]),
            "wv": np.ascontiguousarray(W_qkv[:, 2 * C:][:, cs]),
            "wo": np.ascontiguousarray(W_out[cs, :]),
            "bq": np.ascontiguousarray(b_qkv[cs].reshape(GF, 1)),
            "bk": np.ascontiguousarray(b_qkv[C:][cs].reshape(GF, 1)),
            "bva": bva,
            "mk": mk,
            "ident": ident,
        })
    return in_maps


def run(x, W_qkv, b_qkv, W_out, b_out, trace=False):
    x = np.asarray(x, dtype=np.float32)
    W_qkv = np.asarray(W_qkv, dtype=np.float32)
    b_qkv = np.asarray(b_qkv, dtype=np.float32)
    W_out = np.asarray(W_out, dtype=np.float32)
    b_out = np.asarray(b_out, dtype=np.float32)

    nc = _get_program()
    in_maps = _make_in_maps(x, W_qkv, b_qkv, W_out, b_out)
    res = run_bass_kernel_spmd(nc, in_maps, core_ids=list(range(NCORES)),
                               trace=trace)

    y = np.empty((B, T, C), dtype=np.float32)
    k = np.empty((B, NH, T, HD), dtype=np.float32)
    v = np.empty((B, NH, T, HD), dtype=np.float32)
    for b in range(B):
        r0, r1 = res.results[2 * b], res.results[2 * b + 1]
        y[b] = r0["yp"] + r1["yp"] + b_out[None, :]
        for g, r in ((0, r0), (1, r1)):
            k[b, g * HPC:(g + 1) * HPC] = np.moveaxis(
                r["ko"].reshape(T, HPC, HD), 1, 0)
            v[b, g * HPC:(g + 1) * HPC] = np.moveaxis(
                r["vo"].reshape(T, HPC, HD), 1, 0)
    return (y, k, v), res


def kernel(x, W_qkv, b_qkv, W_out, b_out):
    (y, k, v), _ = run(x, W_qkv, b_qkv, W_out, b_out, trace=False)
    return (y, k, v)
